# revision 1
# baseline (speedup 1.0000x reference)
"""Trainium2 Bass kernel for a linear-chain CRF negative log-likelihood.

Problem: S=32768 sequence steps, L=512 tags.
  loss = logsumexp over all paths (forward algorithm) - gold path score.

Algorithm (device):
  In exp-space the forward recurrence is LINEAR: w_{t} = D_t E w_{t-1}
  with E = exp(T) constant and D_t = diag(exp(logit[t])).  Products of
  positive matrices contract to rank-1 at ~0.06/step, so the 32767-step
  serial chain is split into 2048 segments of 16 transitions.  For each
  segment we compute g = M_seg @ 1 (forward chain, all-ones init) and
  h = M_seg^T @ 1 (backward chain).  Host stitches exactly in float64:
      alpha_end = log g + kappa*n + lse(log h + alpha_start) - lse(log g)
  which is exact up to the rank-1 residual (~0.06^16 ~ 1e-20).
  Each of the 8 cores runs its 256 segments as ONE batch: 16 lockstep
  wall-steps of 32 matmuls ([128,128] bf16 blocks of E) + 2 vector mults.

  Gold score on device: emissions via mask (iota==label) * logit with a
  fused tensor_tensor_reduce over the transposed logit slices;
  transitions via a one-hot count matrix C = Onehot_cur^T @ Onehot_prev
  accumulated in PSUM over 32 matmuls, then sum(C * T).

  Core 7 has 4095 real transitions; one phantom transition (feat=0) pads
  its last segment and is removed exactly in the host stitch using the
  segment's 15-step forward state plus r[i] = lse_j T[j,i].
"""

import numpy as np
import ml_dtypes

import concourse.bass as bass
import concourse.bacc as bacc
import concourse.tile as tile
import concourse.bass_utils as bass_utils
from concourse import mybir

S, L = 32768, 512
NCORES = 8
SPAN = 4096          # transition columns per core (core 7: 4095 real + 1 phantom)
SEG_N = 16           # transitions per segment
SEG_P = 256          # segments per core
KAPPA = 6.74         # constant log-scale folded into E-hat = exp(T - KAPPA)

F32 = mybir.dt.float32
BF16 = mybir.dt.bfloat16

_CACHE = {}


def _emit_body(tc, io, reps=1, phases=("emis", "chain", "gold")):
    nc = tc.nc
    MULT = mybir.AluOpType.mult
    ADD = mybir.AluOpType.add
    EQ = mybir.AluOpType.is_equal
    EXP = mybir.ActivationFunctionType.Exp

    import contextlib
    ctx = contextlib.ExitStack()
    const = ctx.enter_context(tc.tile_pool(name="const", bufs=1))
    fin = ctx.enter_context(tc.tile_pool(name="fin", bufs=2))
    emask = ctx.enter_context(tc.tile_pool(name="emask", bufs=1))
    scratch = ctx.enter_context(tc.tile_pool(name="scratch", bufs=1))
    nmask = ctx.enter_context(tc.tile_pool(name="nmask", bufs=4))
    states = ctx.enter_context(tc.tile_pool(name="states", bufs=3))
    xs = ctx.enter_context(tc.tile_pool(name="xs", bufs=2))
    outp = ctx.enter_context(tc.tile_pool(name="outp", bufs=1))
    pf_pool = ctx.enter_context(tc.tile_pool(name="pf", bufs=1, space="PSUM"))
    pb_pool = ctx.enter_context(tc.tile_pool(name="pb", bufs=1, space="PSUM"))
    pc_pool = ctx.enter_context(tc.tile_pool(name="pc", bufs=1, space="PSUM"))

    # ---- constants / weights -------------------------------------------
    kbias = const.tile([128, 1], F32, tag="kbias")
    nc.gpsimd.memset(kbias[:], -KAPPA)
    w_f = []   # fwd lhsT chunks: exp(T^T - k) [i-part, j-free]
    w_b = []   # bwd lhsT chunks: exp(T - k)   [j-part, i-free]
    for c in range(4):
        tt = fin.tile([128, 512], F32, tag="fin")
        nc.sync.dma_start(tt[:], io["t_tr"][c * 128:(c + 1) * 128, :])
        wf = const.tile([128, 512], BF16, tag=f"wf{c}")
        nc.scalar.activation(wf[:], tt[:], EXP, bias=kbias[:])
        w_f.append(wf)

        tn = fin.tile([128, 512], F32, tag="fin")
        nc.sync.dma_start(tn[:], io["t_nat"][c * 128:(c + 1) * 128, :])
        wb = const.tile([128, 512], BF16, tag=f"wb{c}")
        nc.scalar.activation(wb[:], tn[:], EXP, bias=kbias[:])
        w_b.append(wb)

    iota_free = const.tile([128, 512], F32, tag="iota_free")
    nc.sync.dma_start(iota_free[:], io["iota_free"][:])
    iota_col = const.tile([128, 4], F32, tag="iota_col")
    nc.sync.dma_start(iota_col[:], io["iota_col"][:])
    lab_c = const.tile([128, 32], F32, tag="lab_c")
    nc.sync.dma_start(lab_c[:], io["lab_c"][:])
    lab_p = const.tile([128, 32], F32, tag="lab_p")
    nc.sync.dma_start(lab_p[:], io["lab_p"][:])
    lab_bc = const.tile([128, SPAN], F32, tag="lab_bc")
    nc.sync.dma_start(lab_bc[:], io["lab_bc"][:])
    ones_col = const.tile([128, 1], F32, tag="ones_col")
    nc.gpsimd.memset(ones_col[:], 1.0)

    # ---- F = exp(logitT) + emission gold (fused over the same chunks) --
    f_all = const.tile([128, 4 * SPAN], F32, tag="f_all")
    emis_ps = pc_pool.tile([1, 512], F32, tag="pc")
    for c in range(4):
        chunk = fin.tile([128, SPAN], F32, tag="fin")
        nc.sync.dma_start(chunk[:], io["logitT"][c * 128:(c + 1) * 128, :])
        nc.scalar.activation(f_all[:, c * SPAN:(c + 1) * SPAN], chunk[:], EXP)
        if "emis" in phases:
            m = emask.tile([128, SPAN], F32, tag="emask")
            nc.vector.tensor_scalar(m[:], lab_bc[:], iota_col[:, c:c + 1], None, op0=EQ)
            sc = scratch.tile([128, SPAN], F32, tag="scratch")
            nc.vector.tensor_mul(sc[:], chunk[:], m[:])
            for q in range(8):
                nc.tensor.matmul(
                    emis_ps[:, :],
                    ones_col[:],
                    sc[:, q * 512:(q + 1) * 512],
                    start=(c == 0 and q == 0), stop=(c == 3 and q == 7))
    emis_sb = outp.tile([1, 512], F32, tag="emis_sb")
    if "emis" in phases:
        nc.vector.tensor_copy(emis_sb[:], emis_ps[:])
    else:
        nc.gpsimd.memset(emis_sb[:], 0.0)
    nc.sync.dma_start(io["emis_out"][:], emis_sb[:])

    f3 = f_all[:].rearrange("p (c t) -> p c t", c=4)

    def f_slice(off):
        # [128, 4, 256] strided view: col = c*SPAN + off + 16*k
        return f3[:, :, off: off + 16 * (SEG_P - 1) + 1: 16]

    for rep in range(reps):
        if "chain" not in phases:
            break
        # ---- chain phase ----------------------------------------------
        psum_f = pf_pool.tile([128, 1024], F32, tag="pf")
        psum_b = pb_pool.tile([128, 1024], F32, tag="pb")
        do_gold = "gold" in phases
        if do_gold:
            psum_c = pc_pool.tile([128, 2048], F32, tag="pc")

        u = states.tile([128, 1024], BF16, tag="u")
        nc.gpsimd.memset(u[:], 1.0)
        x = xs.tile([128, 1024], BF16, tag="x")
        # bwd step 0 pre-mult state: X0 = F at local offset 15 (ones * F)
        nc.vector.tensor_copy(x[:].rearrange("p (c k) -> p c k", c=4), f_slice(SEG_N - 1))

        def gold_w(w):
            # one 128-row tile of the transition count matmuls; masks on
            # GPSIMD so the DVE stays free for the chain multiplies
            mc = nmask.tile([128, 512], BF16, tag="mc")
            nc.gpsimd.tensor_scalar(mc[:], iota_free[:], lab_c[:, w:w + 1], None, op0=EQ)
            mp = nmask.tile([128, 512], BF16, tag="mp")
            nc.gpsimd.tensor_scalar(mp[:], iota_free[:], lab_p[:, w:w + 1], None, op0=EQ)
            for q in range(4):
                nc.tensor.matmul(
                    psum_c[:, q * 512:(q + 1) * 512],
                    mc[:, q * 128:(q + 1) * 128],
                    mp[:],
                    start=(w == 0), stop=(w == 31))

        u_prev = None
        for s in range(SEG_N):
            if do_gold:
                gold_w(2 * s)
                gold_w(2 * s + 1)
            # fwd: psum_f[jc*256+k] = sum_ic Wf[ic][:,jc]^T @ u[ic*256+k]
            for jc in range(4):
                for ic in range(4):
                    nc.tensor.matmul(
                        psum_f[:, jc * 256:(jc + 1) * 256],
                        w_f[ic][:, jc * 128:(jc + 1) * 128],
                        u[:, ic * 256:(ic + 1) * 256],
                        start=(ic == 0), stop=(ic == 3))
            # bwd: psum_b[ic*256+k] = sum_jc Wb[jc][:,ic]^T @ x[jc*256+k]
            for ic in range(4):
                for jc in range(4):
                    nc.tensor.matmul(
                        psum_b[:, ic * 256:(ic + 1) * 256],
                        w_b[jc][:, ic * 128:(ic + 1) * 128],
                        x[:, jc * 256:(jc + 1) * 256],
                        start=(jc == 0), stop=(jc == 3))
            # fwd elementwise: u' = psum_f * F[., s+1 + 16k]  (local offset s)
            u_prev = u
            u = states.tile([128, 1024], BF16, tag="u")
            nc.vector.tensor_mul(
                u[:].rearrange("p (c k) -> p c k", c=4),
                psum_f[:].rearrange("p (c k) -> p c k", c=4),
                f_slice(s))
            if s == SEG_N - 2:
                nc.sync.dma_start(io["gp_out"][:], u[:])
            # bwd elementwise: x' = psum_b * F[., 15-s-1 ...]
            if s < SEG_N - 1:
                x = xs.tile([128, 1024], BF16, tag="x")
                nc.vector.tensor_mul(
                    x[:].rearrange("p (c k) -> p c k", c=4),
                    psum_b[:].rearrange("p (c k) -> p c k", c=4),
                    f_slice(SEG_N - 2 - s))
            else:
                h_sb = outp.tile([128, 1024], BF16, tag="h_sb")
                nc.vector.tensor_copy(h_sb[:], psum_b[:])
                nc.sync.dma_start(io["h_out"][:], h_sb[:])
        nc.sync.dma_start(io["g_out"][:], u[:])

        if "gold" not in phases:
            continue
        # ---- transition gold reduce: sum(C*T)
        trans_acc = outp.tile([128, 4], F32, tag="trans_acc")
        for q in range(4):
            tn = fin.tile([128, 512], F32, tag="fin")
            nc.sync.dma_start(tn[:], io["t_nat"][q * 128:(q + 1) * 128, :])
            sc = scratch.tile([128, 512], F32, tag="scratch2")
            nc.vector.tensor_mul(sc[:], psum_c[:, q * 512:(q + 1) * 512], tn[:])
            nc.vector.tensor_reduce(
                trans_acc[:, q:q + 1], sc[:], axis=mybir.AxisListType.X, op=ADD)
        nc.sync.dma_start(io["trans_out"][:], trans_acc[:])

    ctx.close()


def build_program(reps=1, phases=("emis", "chain", "gold")):
    nc = bacc.Bacc("TRN2", target_bir_lowering=False, debug=False,
                   num_devices=NCORES)
    io = {}
    def inp(name, shape, dt=F32):
        io[name] = nc.dram_tensor(name, shape, dt, kind="ExternalInput").ap()
    def outp(name, shape, dt):
        io[name] = nc.dram_tensor(name, shape, dt, kind="ExternalOutput").ap()

    inp("logitT", [L, SPAN])
    inp("t_nat", [L, L])
    inp("t_tr", [L, L])
    inp("lab_bc", [128, SPAN])
    inp("lab_c", [128, 32])
    inp("lab_p", [128, 32])
    inp("iota_free", [128, 512])
    inp("iota_col", [128, 4])
    outp("g_out", [128, 1024], BF16)
    outp("gp_out", [128, 1024], BF16)
    outp("h_out", [128, 1024], BF16)
    outp("emis_out", [1, 512], F32)
    outp("trans_out", [128, 4], F32)

    with tile.TileContext(nc) as tc:
        _emit_body(tc, io, reps=reps, phases=phases)
    nc.compile()
    return nc


def make_in_maps(logit, labels, T):
    """Host-side sharding/layout prep. logit [S,L] f32, labels [S] int, T [L,L] f32."""
    logit = np.asarray(logit, dtype=np.float32)
    labels = np.asarray(labels).astype(np.int64)
    T = np.asarray(T, dtype=np.float32)

    logitT_full = np.ascontiguousarray(logit.T)          # [L, S]
    t_nat = np.ascontiguousarray(T)
    t_tr = np.ascontiguousarray(T.T)
    iota_free = np.tile(np.arange(512, dtype=np.float32), (128, 1))
    iota_col = (np.arange(128, dtype=np.float32)[:, None]
                + 128.0 * np.arange(4, dtype=np.float32)[None, :])
    iota_col = np.ascontiguousarray(iota_col)

    in_maps = []
    for c in range(NCORES):
        t0 = c * SPAN + 1                     # first transition of this core
        sl = np.zeros((L, SPAN), dtype=np.float32)
        lr = np.full((1, SPAN), -1.0, dtype=np.float32)
        lc = np.full((128, 32), -1.0, dtype=np.float32)
        lp = np.full((128, 32), -2.0, dtype=np.float32)
        n_real = min(SPAN, S - t0)            # 4096, core 7: 4095
        sl[:, :n_real] = logitT_full[:, t0:t0 + n_real]
        lr[0, :n_real] = labels[t0:t0 + n_real].astype(np.float32)
        lab_c_flat = np.full(SPAN, -1.0, dtype=np.float32)
        lab_p_flat = np.full(SPAN, -2.0, dtype=np.float32)
        lab_c_flat[:n_real] = labels[t0:t0 + n_real].astype(np.float32)
        lab_p_flat[:n_real] = labels[t0 - 1:t0 - 1 + n_real].astype(np.float32)
        # [p, w] layout with t = t0 + w*128 + p
        lc[:, :] = lab_c_flat.reshape(32, 128).T
        lp[:, :] = lab_p_flat.reshape(32, 128).T
        in_maps.append({
            "logitT": sl,
            "t_nat": t_nat,
            "t_tr": t_tr,
            "lab_bc": np.ascontiguousarray(np.tile(lr, (128, 1))),
            "lab_c": np.ascontiguousarray(lc),
            "lab_p": np.ascontiguousarray(lp),
            "iota_free": iota_free,
            "iota_col": iota_col,
        })
    return in_maps


def _lse(x, axis=None):
    m = np.max(x, axis=axis, keepdims=True)
    out = m + np.log(np.sum(np.exp(x - m), axis=axis, keepdims=True))
    return np.squeeze(out, axis=axis) if axis is not None else out.reshape(())


def host_stitch(results, logit, labels, T):
    """Combine per-core segment chain outputs into the scalar loss (float64)."""
    logit64 = np.asarray(logit, dtype=np.float64)
    T64 = np.asarray(T, dtype=np.float64)
    labels = np.asarray(labels).astype(np.int64)

    def vecs(arr):
        # [128, 1024] bf16 -> [512, 256] float64 (tag, segment)
        a = np.asarray(arr).astype(np.float64).reshape(128, 4, SEG_P)
        return a.transpose(1, 0, 2).reshape(L, SEG_P)

    r_corr = None
    with np.errstate(divide="ignore"):
        alpha = logit64[0].copy()
        for c in range(NCORES):
            g = np.log(vecs(results[c]["g_out"]))
            gp = np.log(vecs(results[c]["gp_out"]))
            h = np.log(vecs(results[c]["h_out"]))
            for k in range(SEG_P):
                phantom = (c == NCORES - 1 and k == SEG_P - 1)
                if not phantom:
                    alpha = (g[:, k] + KAPPA * SEG_N
                             + _lse(h[:, k] + alpha) - _lse(g[:, k]))
                else:
                    if r_corr is None:
                        r_corr = _lse(T64, axis=0)   # r[i] = lse_j T[j,i]
                    alpha = (gp[:, k] + KAPPA * SEG_N
                             + _lse(h[:, k] + alpha) - _lse(gp[:, k] + r_corr))
        log_z = _lse(alpha)

    emis = sum(float(np.asarray(results[c]["emis_out"], dtype=np.float64).sum())
               for c in range(NCORES))
    trans = sum(float(np.asarray(results[c]["trans_out"], dtype=np.float64).sum())
                for c in range(NCORES))
    gold = float(logit64[0, labels[0]]) + emis + trans
    return float(log_z) - gold


def kernel(logit, labels, T):
    key = "prog"
    if key not in _CACHE:
        _CACHE[key] = build_program()
    nc = _CACHE[key]
    in_maps = make_in_maps(logit, labels, T)
    res = bass_utils.run_bass_kernel_spmd(nc, in_maps, core_ids=list(range(NCORES)))
    loss = host_stitch(res.results, logit, labels, T)
    return np.array(loss, dtype=np.float32)



# revision 4
# speedup vs baseline: 14.2500x; 14.2500x over previous
"""Trainium2 Bass kernel for a linear-chain CRF negative log-likelihood.

Problem: S=32768 sequence steps, L=512 tags.
  loss = logsumexp over all paths (forward algorithm) - gold path score.

Algorithm (device):
  In exp-space the forward recurrence is LINEAR: w_t = D_t E w_{t-1}
  with E = exp(T) constant and D_t = diag(exp(logit[t])).  Products of
  positive matrices contract to rank-1 extremely fast, so the 32767-step
  serial chain is split into 2048 segments of 16 transitions.  For each
  segment the device computes g = M_seg @ 1 (forward chain from ones).
  Writing M_seg ~= sigma a b^T (rank-1), g carries sigma and the
  direction a; the direction b is recovered from a 1-step truncated
  backward chain h_hat = E^T f_0 (per segment), which is accurate to the
  product's second-singular-value ratio (~1e-3).  Host stitches in
  float64 with the scale-invariant formula
      alpha_end = log g + kappa*n + lse(log h_hat + alpha_start)
                  - lse(log h_hat)
  which needs only h_hat's DIRECTION, so the 15 remaining backward steps
  are never computed.  The gold path score is a trivial host-side gather.

  Device work per core = 256 segments x 16 forward steps + 1 backward
  step.  Each wall-step: 16 matmuls ([128,128] bf16 blocks of E applied
  to 4x[128,256] state chunks) + 4 chunk-wise D_t multiplies on DVE.
  Four separate PSUM tiles (x2 ping-pong) let the DVE multiply of chunk
  c start as soon as chunk c's accumulation group retires, overlapping
  DVE with the tensor engine; the tensor engine is the bottleneck.

  Core 7 has 4095 real transitions; one phantom transition (feat=0) pads
  its last segment and is removed exactly in the host stitch by using
  the segment's 15-step forward state gp with kappa*15.
"""

import numpy as np
import ml_dtypes

import concourse.bass as bass
import concourse.bacc as bacc
import concourse.tile as tile
import concourse.bass_utils as bass_utils
from concourse import mybir

S, L = 32768, 512
NCORES = 8
SPAN = 4096          # transition columns per core (core 7: 4095 real + 1 phantom)
SEG_N = 16           # transitions per segment
SEG_P = 256          # segments per core
KAPPA = 6.74         # constant log-scale folded into E-hat = exp(T - KAPPA)

F32 = mybir.dt.float32
BF16 = mybir.dt.bfloat16

_CACHE = {}


def _emit_body(tc, io, reps=1, loop=False):
    nc = tc.nc
    EXP = mybir.ActivationFunctionType.Exp

    import contextlib
    ctx = contextlib.ExitStack()
    const = ctx.enter_context(tc.tile_pool(name="const", bufs=1))
    fin = ctx.enter_context(tc.tile_pool(name="fin", bufs=2))
    ustates = ctx.enter_context(tc.tile_pool(name="ustates", bufs=2))
    xs = ctx.enter_context(tc.tile_pool(name="xs", bufs=1))
    outp = ctx.enter_context(tc.tile_pool(name="outp", bufs=1))
    pf_pool = ctx.enter_context(tc.tile_pool(name="pf", bufs=1, space="PSUM"))
    pb_pool = ctx.enter_context(tc.tile_pool(name="pb", bufs=1, space="PSUM"))

    # ---- constants / weights -------------------------------------------
    kbias = const.tile([128, 1], F32, tag="kbias")
    nc.gpsimd.memset(kbias[:], -KAPPA)
    w_f = []   # fwd lhsT chunks: exp(T^T - k) [i-part, j-free]
    w_b = []   # bwd lhsT chunks: exp(T - k)   [j-part, i-free]
    for c in range(4):
        tt = fin.tile([128, 512], F32, tag="tstage")
        nc.sync.dma_start(tt[:], io["t_tr"][c * 128:(c + 1) * 128, :])
        wf = const.tile([128, 512], BF16, tag=f"wf{c}")
        nc.scalar.activation(wf[:], tt[:], EXP, bias=kbias[:])
        w_f.append(wf)

        tn = fin.tile([128, 512], F32, tag="tstage")
        nc.sync.dma_start(tn[:], io["t_nat"][c * 128:(c + 1) * 128, :])
        wb = const.tile([128, 512], BF16, tag=f"wb{c}")
        nc.scalar.activation(wb[:], tn[:], EXP, bias=kbias[:])
        w_b.append(wb)

    # ---- F = exp(logitT) ------------------------------------------------
    f_all = const.tile([128, 4 * SPAN], F32, tag="f_all")
    for c in range(4):
        chunk = fin.tile([128, SPAN], F32, tag="fstage")
        nc.sync.dma_start(chunk[:], io["logitT"][c * 128:(c + 1) * 128, :])
        nc.scalar.activation(f_all[:, c * SPAN:(c + 1) * SPAN], chunk[:], EXP)

    def f_c(c, s):
        # [128, 256] strided view of chunk c, local step s: col = c*SPAN + s + 16*k
        off = c * SPAN + s
        return f_all[:, off: off + 16 * (SEG_P - 1) + 1: 16]

    def emit_rep():
        # init forward states to ones; stage backward seed x = f_0 chunks
        u = []
        for c in range(4):
            t = ustates.tile([128, SEG_P], BF16, tag=f"u{c}", name=f"u{c}")
            nc.gpsimd.memset(t[:], 1.0)
            u.append(t)
        x = []
        for c in range(4):
            t = xs.tile([128, SEG_P], BF16, tag=f"x{c}", name=f"x{c}")
            nc.gpsimd.tensor_copy(t[:], f_c(c, 0))
            x.append(t)

        for s in range(SEG_N):
            ps = [pf_pool.tile([128, SEG_P], F32, tag=f"pf{jc}", name=f"pf{jc}")
                  for jc in range(4)]
            for jc in range(4):
                for ic in range(4):
                    nc.tensor.matmul(
                        ps[jc][:],
                        w_f[ic][:, jc * 128:(jc + 1) * 128],
                        u[ic][:],
                        start=(ic == 0), stop=(ic == 3))
            u_new = []
            for c in range(4):
                t = ustates.tile([128, SEG_P], BF16, tag=f"u{c}", name=f"u{c}")
                nc.vector.tensor_mul(t[:], ps[c][:], f_c(c, s))
                u_new.append(t)
                if s == SEG_N - 2:
                    nc.sync.dma_start(io["gp_out"][:, c * SEG_P:(c + 1) * SEG_P], t[:])
                elif s == SEG_N - 1:
                    nc.sync.dma_start(io["g_out"][:, c * SEG_P:(c + 1) * SEG_P], t[:])
            u = u_new

        # backward: one E^T application of x = f_0
        pb_t = pb_pool.tile([128, 4 * SEG_P], F32, tag="pb")
        for ic in range(4):
            for jc in range(4):
                nc.tensor.matmul(
                    pb_t[:, ic * SEG_P:(ic + 1) * SEG_P],
                    w_b[jc][:, ic * 128:(ic + 1) * 128],
                    x[jc][:],
                    start=(jc == 0), stop=(jc == 3))
        h_sb = outp.tile([128, 4 * SEG_P], BF16, tag="h_sb")
        nc.vector.tensor_copy(h_sb[:], pb_t[:])
        nc.sync.dma_start(io["h_out"][:], h_sb[:])

    if loop:
        with tc.For_i(0, reps, 1):
            emit_rep()
    else:
        for _ in range(reps):
            emit_rep()

    ctx.close()


def build_program(reps=1, loop=False):
    nc = bacc.Bacc("TRN2", target_bir_lowering=False, debug=False,
                   num_devices=NCORES)
    io = {}
    def inp(name, shape, dt=F32):
        io[name] = nc.dram_tensor(name, shape, dt, kind="ExternalInput").ap()
    def outp(name, shape, dt):
        io[name] = nc.dram_tensor(name, shape, dt, kind="ExternalOutput").ap()

    inp("logitT", [L, SPAN])
    inp("t_nat", [L, L])
    inp("t_tr", [L, L])
    outp("g_out", [128, 1024], BF16)
    outp("gp_out", [128, 1024], BF16)
    outp("h_out", [128, 1024], BF16)

    with tile.TileContext(nc) as tc:
        _emit_body(tc, io, reps=reps, loop=loop)
    nc.compile()
    return nc


def make_in_maps(logit, labels, T):
    """Host-side sharding/layout prep. logit [S,L] f32, labels [S] int, T [L,L] f32."""
    logit = np.asarray(logit, dtype=np.float32)
    T = np.asarray(T, dtype=np.float32)

    logitT_full = np.ascontiguousarray(logit.T)          # [L, S]
    t_nat = np.ascontiguousarray(T)
    t_tr = np.ascontiguousarray(T.T)

    in_maps = []
    for c in range(NCORES):
        t0 = c * SPAN + 1                     # first transition of this core
        sl = np.zeros((L, SPAN), dtype=np.float32)
        n_real = min(SPAN, S - t0)            # 4096, core 7: 4095
        sl[:, :n_real] = logitT_full[:, t0:t0 + n_real]
        in_maps.append({
            "logitT": sl,
            "t_nat": t_nat,
            "t_tr": t_tr,
        })
    return in_maps


def _lse(x, axis=None):
    m = np.max(x, axis=axis, keepdims=True)
    out = m + np.log(np.sum(np.exp(x - m), axis=axis, keepdims=True))
    return np.squeeze(out, axis=axis) if axis is not None else out.reshape(())


def host_stitch(results, logit, labels, T):
    """Combine per-core segment chain outputs into the scalar loss (float64)."""
    logit64 = np.asarray(logit, dtype=np.float64)
    T64 = np.asarray(T, dtype=np.float64)
    labels = np.asarray(labels).astype(np.int64)

    def vecs(arr):
        # [128, 1024] bf16 -> [512, 256] float64 (tag, segment)
        a = np.asarray(arr).astype(np.float64).reshape(128, 4, SEG_P)
        return a.transpose(1, 0, 2).reshape(L, SEG_P)

    with np.errstate(divide="ignore"):
        alpha = logit64[0].copy()
        for c in range(NCORES):
            g = np.log(vecs(results[c]["g_out"]))
            gp = np.log(vecs(results[c]["gp_out"]))
            h = np.log(vecs(results[c]["h_out"]))
            for k in range(SEG_P):
                phantom = (c == NCORES - 1 and k == SEG_P - 1)
                if phantom:
                    logg = gp[:, k] + KAPPA * (SEG_N - 1)
                else:
                    logg = g[:, k] + KAPPA * SEG_N
                alpha = logg + _lse(h[:, k] + alpha) - _lse(h[:, k])
        log_z = _lse(alpha)

    gold = (logit64[np.arange(S), labels].sum()
            + T64[labels[1:], labels[:-1]].sum())
    return float(log_z) - gold


def kernel(logit, labels, T):
    key = "prog"
    if key not in _CACHE:
        _CACHE[key] = build_program()
    nc = _CACHE[key]
    in_maps = make_in_maps(logit, labels, T)
    res = bass_utils.run_bass_kernel_spmd(nc, in_maps, core_ids=list(range(NCORES)))
    loss = host_stitch(res.results, logit, labels, T)
    return np.array(loss, dtype=np.float32)


# revision 5
# speedup vs baseline: 15.2776x; 1.0721x over previous
"""Trainium2 Bass kernel for a linear-chain CRF negative log-likelihood.

Problem: S=32768 sequence steps, L=512 tags.
  loss = logsumexp over all paths (forward algorithm) - gold path score.

Algorithm (device):
  In exp-space the forward recurrence is LINEAR: w_t = D_t E w_{t-1}
  with E = exp(T) constant and D_t = diag(exp(logit[t])).  Products of
  positive matrices contract to rank-1 extremely fast, so the 32767-step
  serial chain is split into 2048 segments of 16 transitions.  For each
  segment the device computes g = M_seg @ 1 (forward chain from ones).
  Writing M_seg ~= sigma a b^T (rank-1), g carries sigma and the
  direction a; the direction b is recovered from a 1-step truncated
  backward chain h_hat = E^T f_0 (per segment), which is accurate to the
  product's second-singular-value ratio (~1e-3).  Host stitches in
  float64 with the scale-invariant formula
      alpha_end = log g + kappa*n + lse(log h_hat + alpha_start)
                  - lse(log h_hat)
  which needs only h_hat's DIRECTION, so the 15 remaining backward steps
  are never computed.  The gold path score is a trivial host-side gather.

  Device work per core = 256 segments x 16 forward steps + 1 backward
  step.  Each wall-step: 16 matmuls ([128,128] bf16 blocks of E applied
  to 4x[128,256] state chunks) + 4 chunk-wise D_t multiplies on DVE.
  Four separate PSUM tiles (x2 ping-pong) let the DVE multiply of chunk
  c start as soon as chunk c's accumulation group retires, overlapping
  DVE with the tensor engine; the tensor engine is the bottleneck.

  Core 7 has 4095 real transitions; one phantom transition (feat=0) pads
  its last segment and is removed exactly in the host stitch by using
  the segment's 15-step forward state gp with kappa*15.
"""

import numpy as np
import ml_dtypes

import concourse.bass as bass
import concourse.bacc as bacc
import concourse.tile as tile
import concourse.bass_utils as bass_utils
from concourse import mybir

S, L = 32768, 512
NCORES = 8
SPAN = 4096          # transition columns per core (core 7: 4095 real + 1 phantom)
SEG_N = 8            # transitions per segment
SEG_P = 512          # segments per core
KAPPA = 6.74         # constant log-scale folded into E-hat = exp(T - KAPPA)

F32 = mybir.dt.float32
BF16 = mybir.dt.bfloat16

_CACHE = {}


def _emit_body(tc, io, reps=1, loop=False):
    nc = tc.nc
    EXP = mybir.ActivationFunctionType.Exp

    import contextlib
    ctx = contextlib.ExitStack()
    const = ctx.enter_context(tc.tile_pool(name="const", bufs=1))
    fin = ctx.enter_context(tc.tile_pool(name="fin", bufs=2))
    ustates = ctx.enter_context(tc.tile_pool(name="ustates", bufs=2))
    xs = ctx.enter_context(tc.tile_pool(name="xs", bufs=1))
    outp = ctx.enter_context(tc.tile_pool(name="outp", bufs=1))
    pf_pool = ctx.enter_context(tc.tile_pool(name="pf", bufs=1, space="PSUM"))
    pb_pool = ctx.enter_context(tc.tile_pool(name="pb", bufs=1, space="PSUM"))

    # ---- constants / weights -------------------------------------------
    kbias = const.tile([128, 1], F32, tag="kbias")
    nc.gpsimd.memset(kbias[:], -KAPPA)
    w_f = []   # fwd lhsT chunks: exp(T^T - k) [i-part, j-free]
    w_b = []   # bwd lhsT chunks: exp(T - k)   [j-part, i-free]
    for c in range(4):
        tt = fin.tile([128, 512], F32, tag="tstage")
        nc.sync.dma_start(tt[:], io["t_tr"][c * 128:(c + 1) * 128, :])
        wf = const.tile([128, 512], BF16, tag=f"wf{c}")
        nc.scalar.activation(wf[:], tt[:], EXP, bias=kbias[:])
        w_f.append(wf)

        tn = fin.tile([128, 512], F32, tag="tstage")
        nc.sync.dma_start(tn[:], io["t_nat"][c * 128:(c + 1) * 128, :])
        wb = const.tile([128, 512], BF16, tag=f"wb{c}")
        nc.scalar.activation(wb[:], tn[:], EXP, bias=kbias[:])
        w_b.append(wb)

    # ---- F = exp(logitT) ------------------------------------------------
    f_all = const.tile([128, 4 * SPAN], F32, tag="f_all")
    for c in range(4):
        chunk = fin.tile([128, SPAN], F32, tag="fstage")
        nc.sync.dma_start(chunk[:], io["logitT"][c * 128:(c + 1) * 128, :])
        nc.scalar.activation(f_all[:, c * SPAN:(c + 1) * SPAN], chunk[:], EXP)

    def f_c(c, s):
        # [128, SEG_P] strided view of chunk c, local step s: col = c*SPAN + s + SEG_N*k
        off = c * SPAN + s
        return f_all[:, off: off + SEG_N * (SEG_P - 1) + 1: SEG_N]

    def emit_rep():
        # init forward states to ones; stage backward seed x = f_0 chunks
        u = []
        for c in range(4):
            t = ustates.tile([128, SEG_P], BF16, tag=f"u{c}", name=f"u{c}")
            nc.gpsimd.memset(t[:], 1.0)
            u.append(t)
        x = []
        for c in range(4):
            t = xs.tile([128, SEG_P], BF16, tag=f"x{c}", name=f"x{c}")
            nc.gpsimd.tensor_copy(t[:], f_c(c, 0))
            x.append(t)

        for s in range(SEG_N):
            ps = [pf_pool.tile([128, SEG_P], F32, tag=f"pf{jc}", name=f"pf{jc}")
                  for jc in range(4)]
            for jc in range(4):
                for ic in range(4):
                    nc.tensor.matmul(
                        ps[jc][:],
                        w_f[ic][:, jc * 128:(jc + 1) * 128],
                        u[ic][:],
                        start=(ic == 0), stop=(ic == 3))
            u_new = []
            for c in range(4):
                t = ustates.tile([128, SEG_P], BF16, tag=f"u{c}", name=f"u{c}")
                nc.vector.tensor_mul(t[:], ps[c][:], f_c(c, s))
                u_new.append(t)
                if s == SEG_N - 2:
                    nc.sync.dma_start(io["gp_out"][:, c * SEG_P:(c + 1) * SEG_P], t[:])
                elif s == SEG_N - 1:
                    nc.sync.dma_start(io["g_out"][:, c * SEG_P:(c + 1) * SEG_P], t[:])
            u = u_new

        # backward: one E^T application of x = f_0
        pb_t = pb_pool.tile([128, 4 * SEG_P], F32, tag="pb")
        for ic in range(4):
            for jc in range(4):
                nc.tensor.matmul(
                    pb_t[:, ic * SEG_P:(ic + 1) * SEG_P],
                    w_b[jc][:, ic * 128:(ic + 1) * 128],
                    x[jc][:],
                    start=(jc == 0), stop=(jc == 3))
        h_sb = outp.tile([128, 4 * SEG_P], BF16, tag="h_sb")
        nc.vector.tensor_copy(h_sb[:], pb_t[:])
        nc.sync.dma_start(io["h_out"][:], h_sb[:])

    if loop:
        with tc.For_i(0, reps, 1):
            emit_rep()
    else:
        for _ in range(reps):
            emit_rep()

    ctx.close()


def build_program(reps=1, loop=False):
    nc = bacc.Bacc("TRN2", target_bir_lowering=False, debug=False,
                   num_devices=NCORES)
    io = {}
    def inp(name, shape, dt=F32):
        io[name] = nc.dram_tensor(name, shape, dt, kind="ExternalInput").ap()
    def outp(name, shape, dt):
        io[name] = nc.dram_tensor(name, shape, dt, kind="ExternalOutput").ap()

    inp("logitT", [L, SPAN])
    inp("t_nat", [L, L])
    inp("t_tr", [L, L])
    outp("g_out", [128, 4 * SEG_P], BF16)
    outp("gp_out", [128, 4 * SEG_P], BF16)
    outp("h_out", [128, 4 * SEG_P], BF16)

    with tile.TileContext(nc) as tc:
        _emit_body(tc, io, reps=reps, loop=loop)
    nc.compile()
    return nc


def make_in_maps(logit, labels, T):
    """Host-side sharding/layout prep. logit [S,L] f32, labels [S] int, T [L,L] f32."""
    logit = np.asarray(logit, dtype=np.float32)
    T = np.asarray(T, dtype=np.float32)

    logitT_full = np.ascontiguousarray(logit.T)          # [L, S]
    t_nat = np.ascontiguousarray(T)
    t_tr = np.ascontiguousarray(T.T)

    in_maps = []
    for c in range(NCORES):
        t0 = c * SPAN + 1                     # first transition of this core
        sl = np.zeros((L, SPAN), dtype=np.float32)
        n_real = min(SPAN, S - t0)            # 4096, core 7: 4095
        sl[:, :n_real] = logitT_full[:, t0:t0 + n_real]
        in_maps.append({
            "logitT": sl,
            "t_nat": t_nat,
            "t_tr": t_tr,
        })
    return in_maps


def _lse(x, axis=None):
    m = np.max(x, axis=axis, keepdims=True)
    out = m + np.log(np.sum(np.exp(x - m), axis=axis, keepdims=True))
    return np.squeeze(out, axis=axis) if axis is not None else out.reshape(())


def host_stitch(results, logit, labels, T):
    """Combine per-core segment chain outputs into the scalar loss (float64)."""
    logit64 = np.asarray(logit, dtype=np.float64)
    T64 = np.asarray(T, dtype=np.float64)
    labels = np.asarray(labels).astype(np.int64)

    def vecs(arr):
        # [128, 4*SEG_P] bf16 -> [L, SEG_P] float64 (tag, segment)
        a = np.asarray(arr).astype(np.float64).reshape(128, 4, SEG_P)
        return a.transpose(1, 0, 2).reshape(L, SEG_P)

    with np.errstate(divide="ignore"):
        alpha = logit64[0].copy()
        for c in range(NCORES):
            g = np.log(vecs(results[c]["g_out"]))
            gp = np.log(vecs(results[c]["gp_out"]))
            h = np.log(vecs(results[c]["h_out"]))
            for k in range(SEG_P):
                phantom = (c == NCORES - 1 and k == SEG_P - 1)
                if phantom:
                    logg = gp[:, k] + KAPPA * (SEG_N - 1)
                else:
                    logg = g[:, k] + KAPPA * SEG_N
                alpha = logg + _lse(h[:, k] + alpha) - _lse(h[:, k])
        log_z = _lse(alpha)

    gold = (logit64[np.arange(S), labels].sum()
            + T64[labels[1:], labels[:-1]].sum())
    return float(log_z) - gold


def kernel(logit, labels, T):
    key = "prog"
    if key not in _CACHE:
        _CACHE[key] = build_program()
    nc = _CACHE[key]
    in_maps = make_in_maps(logit, labels, T)
    res = bass_utils.run_bass_kernel_spmd(nc, in_maps, core_ids=list(range(NCORES)))
    loss = host_stitch(res.results, logit, labels, T)
    return np.array(loss, dtype=np.float32)
